# revision 10
# baseline (speedup 1.0000x reference)
"""LocallyConnected2d kernel for 8 TRN2 NeuronCores (Bass/Tile).

Problem (hardcoded):
  features [32, 64, 64, 64] f32, weights [62, 62, 64, 64, 3, 3] f32,
  bias [62, 62, 64] f32 -> out [32, 64, 62, 62] f32
  out[b,o,h,w] = sum_{c,i,j} x[b,c,h+i,w+j] * W[h,w,o,c,i,j] + bias[h,w,o]

Strategy (v5):
  - Shard over Hout: 8 cores x 8 output rows (bands [0,8,...,48,54], last two
    overlap; host takes canonical rows from each core).
  - Weights stream as fp8 e3m4 (x2 scale, /2 on host) = 1 B/el -> 18.9 MB/core
    with zero padding waste; activations stay bf16 (mixed-dtype matmul: only
    fp32 operands must be paired). PSUM accumulates fp32. rel err ~0.014.
  - fA layout [128=(c | c shifted w+1), w, t, b]. Only the lower half (2.6 MB)
    comes from HBM; the w+1-shifted upper half is derived on-chip with
    SBUF->SBUF DMAs on the (idle) gpsimd SWDGE path, chunked to follow the
    HBM chunks. fA[64:,63] is never read, so it needs no init.
  - Per (hg=4-row half-band, wg=4-w group), outputs in PSUM [128,256]:
    partitions=(4w x 32b) via col tile_position, free=(4j x 64 cout).
    * PSUM is zeroed OFF the PE (ACT copy-from-zeros / DVE memset); all
      matmuls run flags=0 (has_written: accumulate where stale-set onto the
      zeros, overwrite the zeros where clear - correct either way). No K=1
      zeroing matmuls, no start=True anywhere.
    * wr MMs (taps i in {0,1}): stationary fA[:, w0+g, hl+tau] K=128, moving
      N=nv*64, tau=j+r grouped. 24 MMs per (hg,wg).
    * tap i=2 is K=64, tau-grouped, split across TWO psum tiles because one
      accumulation group must not mix PE row-tiles (HW limitation, probed):
      taus {0,1,4,5} via fA lower -> main ps (row tile 0); taus {2,3} via
      the w+1-shifted upper half (same x column) -> psB (row tile 64).
      Both partition halves of wk64 carry real data -> no padding bytes.
    * combine: ACT copies psB -> S slice (bf16), DVE tensor_add(S, ps, S).
  - DMA: fA w-chunk [0:8] first on sync, then 8 wk batch-transfers
    ([128,18432] fp8 = 2.36 MB, first one split so wg0 starts early); fA
    chunks 2-4 + quarter-granularity outS dumps ride the scalar ring.
"""

import numpy as np
import ml_dtypes

BF16 = ml_dtypes.bfloat16
F8E3 = ml_dtypes.float8_e3m4
WSCALE = np.float32(2.0)

B, CIN, COUT = 32, 64, 64
H = W = 64
HOUT = WOUT = 62
NCORES = 8
STARTS = [0, 8, 16, 24, 32, 40, 48, 54]

# tau-group geometry: tau = t - hl in 0..5; valid out-rows j in [jlo, jhi]
TAUS = list(range(6))
JLO = [max(0, t - 2) for t in TAUS]
JHI = [min(3, t) for t in TAUS]
NV = [hi - lo + 1 for lo, hi in zip(JLO, JHI)]            # [1,2,3,3,2,1]
TBASE = [0]
for t in TAUS:
    TBASE.append(TBASE[-1] + 4 * NV[t] * 64)              # wr per-tau base col
WR_COLS = TBASE[-1]                                        # 3072
# K64 split: taus 0,1,4,5 -> lower half (row tile 0, main psum);
#            taus 2,3     -> upper half (row tile 64, psB)
K64_LOW_TAUS = [0, 1, 4, 5]
K64_HIGH_TAUS = [2, 3]
K64LO = {}
off = 0
for t in K64_LOW_TAUS:
    K64LO[t] = off
    off += 4 * NV[t] * 64
K64HI = {}
off = 0
for t in K64_HIGH_TAUS:
    K64HI[t] = off
    off += 4 * NV[t] * 64
K64_COLS = 1536                                            # both halves
WG_COLS = WR_COLS + K64_COLS                               # 4608

_STATE = {}


def _build_program():
    import concourse.tile as tile
    from concourse import bacc, mybir

    bf = mybir.dt.bfloat16
    f8 = mybir.dt.float8e3
    f32 = mybir.dt.float32
    ACT_COPY = mybir.ActivationFunctionType.Copy

    nc = bacc.Bacc(None, target_bir_lowering=False)
    featA = nc.dram_tensor("featA", [64, 64, 10, 32], bf, kind="ExternalInput")
    wk_d = nc.dram_tensor("wk", [8, 128, 4 * WG_COLS], f8, kind="ExternalInput")
    outS = nc.dram_tensor("outS", [2, 128, 4096], bf, kind="ExternalOutput")

    with tile.TileContext(nc) as tc:
        with tc.tile_pool(name="feat", bufs=1) as fpool, \
             tc.tile_pool(name="wk", bufs=3) as wkpool, \
             tc.tile_pool(name="st", bufs=2) as spool, \
             tc.tile_pool(name="ps", bufs=4, space="PSUM") as pspool, \
             tc.tile_pool(name="psb", bufs=4, space="PSUM") as psbpool:
            fA = fpool.tile([128, 64, 10, 32], bf)
            # lower half from HBM: first w-chunk ahead of the weight stream
            # on sync, the rest on scalar
            nc.sync.dma_start(fA[0:64, 0:8], featA[:, 0:8])
            nc.scalar.dma_start(fA[0:64, 8:24], featA[:, 8:24])
            nc.scalar.dma_start(fA[0:64, 24:44], featA[:, 24:44])
            nc.scalar.dma_start(fA[0:64, 44:64], featA[:, 44:64])
            # upper half = lower shifted one w, derived on-chip (SWDGE,
            # SBUF->SBUF), chunked to chase the HBM chunks
            nc.gpsimd.dma_start(fA[64:128, 0:7], fA[0:64, 1:8])
            nc.gpsimd.dma_start(fA[64:128, 7:23], fA[0:64, 8:24])
            nc.gpsimd.dma_start(fA[64:128, 23:43], fA[0:64, 24:44])
            nc.gpsimd.dma_start(fA[64:128, 43:63], fA[0:64, 44:64])
            # zeros for the ACT psum-clearing copies
            zb = fpool.tile([128, 256], bf)
            nc.gpsimd.memset(zb[:], 0.0)
            for hg in range(2):
                hl = 4 * hg
                S = spool.tile([128, 4096], bf)
                for qi in range(4):
                    wk = wkpool.tile([128, 4 * WG_COLS], f8)
                    if hg == 0 and qi == 0:
                        # split the first batch so wg0's weights land sooner
                        nc.sync.dma_start(wk[:, 0:WG_COLS],
                                          wk_d[0][:, 0:WG_COLS])
                        nc.sync.dma_start(wk[:, WG_COLS:4 * WG_COLS],
                                          wk_d[0][:, WG_COLS:4 * WG_COLS])
                    else:
                        nc.sync.dma_start(wk[:], wk_d[hg * 4 + qi])
                    for sub in range(4):
                        wg = 4 * qi + sub
                        w0 = min(4 * wg, 58)  # last group overlaps: w 58..61
                        wr = wk[:, sub * WG_COLS:sub * WG_COLS + WR_COLS]
                        wk64 = wk[:, sub * WG_COLS + WR_COLS:
                                  (sub + 1) * WG_COLS]

                        ps = pspool.tile([128, 256], f32)
                        psB = psbpool.tile([128, 256], f32)
                        # Zero PSUM off the PE (see docstring).
                        nc.scalar.activation(ps[:, :], zb[:, :], ACT_COPY)
                        nc.vector.memset(psB[:, :], 0.0)
                        # taps i in {0,1}: K=128 dual-w stationaries
                        for tau in TAUS:
                            nv, jlo = NV[tau], JLO[tau]
                            for g in range(4):
                                off = TBASE[tau] + g * nv * 64
                                nc.tensor.matmul(
                                    ps[32 * g:32 * g + 32,
                                       64 * jlo:64 * (jlo + nv)],
                                    fA[:, w0 + g, hl + tau, :],
                                    wr[:, off:off + nv * 64],
                                    start=False, stop=False,
                                    skip_group_check=True,
                                    tile_position=(0, 32 * g),
                                )
                        # tap i=2, taus {0,1,4,5}: K=64 lower halves -> main
                        for ti, tau in enumerate(K64_LOW_TAUS):
                            nv, jlo = NV[tau], JLO[tau]
                            for g in range(4):
                                off = K64LO[tau] + g * nv * 64
                                nc.tensor.matmul(
                                    ps[32 * g:32 * g + 32,
                                       64 * jlo:64 * (jlo + nv)],
                                    fA[0:64, w0 + g + 2, hl + tau, :],
                                    wk64[0:64, off:off + nv * 64],
                                    start=False,
                                    stop=(ti == 3 and g == 3),
                                    skip_group_check=True,
                                    tile_position=(0, 32 * g),
                                )
                        # tap i=2, taus {2,3}: K=64 upper halves -> psB
                        for ti, tau in enumerate(K64_HIGH_TAUS):
                            nv, jlo = NV[tau], JLO[tau]
                            for g in range(4):
                                off = K64HI[tau] + g * nv * 64
                                nc.tensor.matmul(
                                    psB[32 * g:32 * g + 32,
                                        64 * jlo:64 * (jlo + nv)],
                                    fA[64:128, w0 + g + 1, hl + tau, :],
                                    wk64[64:128, off:off + nv * 64],
                                    start=False,
                                    stop=(ti == 1 and g == 3),
                                    skip_group_check=True,
                                    tile_position=(64, 32 * g),
                                )
                        sl = S[:, 256 * wg:256 * wg + 256]
                        nc.scalar.activation(sl, psB[:, :], ACT_COPY)
                        nc.vector.tensor_add(sl, ps[:], sl)
                    # quarter-granularity output dump (keeps the tail short)
                    nc.scalar.dma_start(
                        outS[hg][:, 1024 * qi:1024 * (qi + 1)],
                        S[:, 1024 * qi:1024 * (qi + 1)])
    nc.compile()
    return nc


def _get_nc():
    if "nc" not in _STATE:
        _STATE["nc"] = _build_program()
    return _STATE["nc"]


def _quant_w(a):
    return np.clip(a * WSCALE, -15.0, 15.0).astype(F8E3)


def _prep_inputs(features, weights):
    """Build the 8 per-core input dicts (device layouts)."""
    x = np.asarray(features, dtype=np.float32)
    Wt = np.asarray(weights, dtype=np.float32)

    # w-slot -> real w: last group overlaps (w 58..61)
    widx = list(range(60)) + [58, 59, 60, 61]

    in_maps = []
    for s in STARTS:
        xt = x[:, :, s:s + 10, :].transpose(1, 3, 2, 0)    # [c, w, t, b]
        fA = np.ascontiguousarray(xt, dtype=BF16)          # lower half only

        Wb = Wt[s:s + 8]                                   # [8, 62, o, c, 3, 3]
        Wsel = Wb[:, widx]                                 # [8, 64slots, o, c, 3, 3]
        WT = Wsel.transpose(4, 5, 3, 0, 1, 2)              # [r, i, c, 8h, 64w, o]

        wkf = np.zeros((2, 16, 128, WG_COLS), dtype=np.float32)
        # wr: taps (r, i=d); cols per (tau, g): q -> j=jlo+q, r=tau-j
        wr = wkf[:, :, :, 0:WR_COLS]
        for tau in TAUS:
            nv, jlo = NV[tau], JLO[tau]
            view = wr[:, :, :, TBASE[tau]:TBASE[tau + 1]].reshape(
                2, 16, 128, 4, nv, 64)
            for q in range(nv):
                j = jlo + q
                r = tau - j
                for d in range(2):
                    src = WT[r, d].reshape(CIN, 2, 4, 16, 4, COUT)[:, :, j]
                    view[:, :, d * 64:(d + 1) * 64, :, q, :] = \
                        src.transpose(1, 2, 0, 3, 4)       # [hg, wg, c, g, o]
        # wk64: tap i=2; taus {0,1,4,5} at partitions 0:64, {2,3} at 64:128
        wk64 = wkf[:, :, :, WR_COLS:WG_COLS]
        for tau in TAUS:
            nv, jlo = NV[tau], JLO[tau]
            if tau in K64LO:
                p0, cb = 0, K64LO[tau]
            else:
                p0, cb = 64, K64HI[tau]
            view = wk64[:, :, p0:p0 + 64, cb:cb + 4 * nv * 64].reshape(
                2, 16, 64, 4, nv, 64)
            for q in range(nv):
                j = jlo + q
                r = tau - j
                src = WT[r, 2].reshape(CIN, 2, 4, 16, 4, COUT)[:, :, j]
                view[:, :, :, :, q, :] = src.transpose(1, 2, 0, 3, 4)
        # [2, 16, 128, 4608] -> [8(hg*4+qi), 128, 18432]
        wk = _quant_w(wkf).reshape(2, 4, 4, 128, WG_COLS).transpose(
            0, 1, 3, 2, 4).reshape(8, 128, 4 * WG_COLS)
        wk = np.ascontiguousarray(wk)
        in_maps.append({"featA": fA, "wk": wk})
    return in_maps


def _gather(results, bias):
    out = np.zeros((B, COUT, HOUT, WOUT), dtype=np.float32)
    inv = 1.0 / float(WSCALE)
    for core, s in enumerate(STARTS):
        arr = np.asarray(results[core]["outS"]).astype(np.float32) * inv
        # [hg, g, b, wg, j, o] -> [b, o, hg, j, wg, g]
        arr = arr.reshape(2, 4, 32, 16, 4, 64).transpose(2, 5, 0, 4, 3, 1)
        arr = arr.reshape(32, 64, 8, 64)
        out[:, :, s:s + 8, 0:60] = arr[:, :, :, 0:60]
        out[:, :, s:s + 8, 60:62] = arr[:, :, :, 62:64]
    out += np.asarray(bias, dtype=np.float32).transpose(2, 0, 1)[None]
    return out


def _run(in_maps, trace=False, trace_cores=None):
    from concourse.bass_utils import run_bass_kernel_spmd
    nc = _get_nc()
    return run_bass_kernel_spmd(
        nc, in_maps, core_ids=list(range(NCORES)),
        trace=trace, trace_cores=trace_cores,
    )


def kernel(features, weights, bias):
    in_maps = _prep_inputs(features, weights)
    res = _run(in_maps)
    return _gather(res.results, bias)


# revision 11
# speedup vs baseline: 1.1514x; 1.1514x over previous
"""LocallyConnected2d kernel for 8 TRN2 NeuronCores (Bass/Tile).

Problem (hardcoded):
  features [32, 64, 64, 64] f32, weights [62, 62, 64, 64, 3, 3] f32,
  bias [62, 62, 64] f32 -> out [32, 64, 62, 62] f32
  out[b,o,h,w] = sum_{c,i,j} x[b,c,h+i,w+j] * W[h,w,o,c,i,j] + bias[h,w,o]

Strategy (v5):
  - Shard over Hout: 8 cores x 8 output rows (bands [0,8,...,48,54], last two
    overlap; host takes canonical rows from each core).
  - Weights stream as fp8 e3m4 (x2 scale, /2 on host) = 1 B/el -> 18.9 MB/core
    with zero padding waste; activations stay bf16 (mixed-dtype matmul: only
    fp32 operands must be paired). PSUM accumulates fp32. rel err ~0.014.
  - fA layout [128=(c | c shifted w+1), w, t, b]. Only the lower half (2.6 MB)
    comes from HBM; the w+1-shifted upper half is derived on-chip with
    SBUF->SBUF DMAs on the (idle) gpsimd SWDGE path, chunked to follow the
    HBM chunks. fA[64:,63] is never read, so it needs no init.
  - Per (hg=4-row half-band, wg=4-w group), outputs in PSUM [128,256]:
    partitions=(4w x 32b) via col tile_position, free=(4j x 64 cout).
    * PSUM is zeroed OFF the PE (ACT copy-from-zeros / DVE memset); all
      matmuls run flags=0 (has_written: accumulate where stale-set onto the
      zeros, overwrite the zeros where clear - correct either way). No K=1
      zeroing matmuls, no start=True anywhere.
    * wr MMs (taps i in {0,1}): stationary fA[:, w0+g, hl+tau] K=128, moving
      N=nv*64, tau=j+r grouped. 24 MMs per (hg,wg).
    * tap i=2 is K=64, tau-grouped, split across TWO psum tiles because one
      accumulation group must not mix PE row-tiles (HW limitation, probed):
      taus {0,1,4,5} via fA lower -> main ps (row tile 0); taus {2,3} via
      the w+1-shifted upper half (same x column) -> psB (row tile 64).
      Both partition halves of wk64 carry real data -> no padding bytes.
    * combine: ACT copies psB -> S slice (bf16), DVE tensor_add(S, ps, S).
  - DMA: fA w-chunk [0:8] first on sync, then 8 wk batch-transfers
    ([128,18432] fp8 = 2.36 MB, first one split so wg0 starts early); fA
    chunks 2-4 + quarter-granularity outS dumps ride the scalar ring.
"""

import numpy as np
import ml_dtypes

BF16 = ml_dtypes.bfloat16
F8E3 = ml_dtypes.float8_e3m4
WSCALE = np.float32(2.0)

B, CIN, COUT = 32, 64, 64
H = W = 64
HOUT = WOUT = 62
NCORES = 8
STARTS = [0, 8, 16, 24, 32, 40, 48, 54]

# tau-group geometry: tau = t - hl in 0..5; valid out-rows j in [jlo, jhi]
TAUS = list(range(6))
JLO = [max(0, t - 2) for t in TAUS]
JHI = [min(3, t) for t in TAUS]
NV = [hi - lo + 1 for lo, hi in zip(JLO, JHI)]            # [1,2,3,3,2,1]
TBASE = [0]
for t in TAUS:
    TBASE.append(TBASE[-1] + 4 * NV[t] * 64)              # wr per-tau base col
WR_COLS = TBASE[-1]                                        # 3072
# K64 split: taus 0,1,4,5 -> lower half (row tile 0, main psum);
#            taus 2,3     -> upper half (row tile 64, psB)
K64_LOW_TAUS = [0, 1, 4, 5]
K64_HIGH_TAUS = [2, 3]
K64LO = {}
off = 0
for t in K64_LOW_TAUS:
    K64LO[t] = off
    off += 4 * NV[t] * 64
K64HI = {}
off = 0
for t in K64_HIGH_TAUS:
    K64HI[t] = off
    off += 4 * NV[t] * 64
K64_COLS = 1536                                            # both halves
WG_COLS = WR_COLS + K64_COLS                               # 4608

_STATE = {}


def _build_program():
    import concourse.tile as tile
    from concourse import bacc, mybir

    bf = mybir.dt.bfloat16
    f8 = mybir.dt.float8e3
    f32 = mybir.dt.float32
    ACT_COPY = mybir.ActivationFunctionType.Copy

    nc = bacc.Bacc(None, target_bir_lowering=False)
    featA = nc.dram_tensor("featA", [128, 64, 10, 32], bf, kind="ExternalInput")
    wk_d = nc.dram_tensor("wk", [8, 128, 4 * WG_COLS], f8, kind="ExternalInput")
    outS = nc.dram_tensor("outS", [2, 128, 4096], bf, kind="ExternalOutput")

    with tile.TileContext(nc) as tc:
        with tc.tile_pool(name="feat", bufs=1) as fpool, \
             tc.tile_pool(name="wk", bufs=4) as wkpool, \
             tc.tile_pool(name="st", bufs=2) as spool, \
             tc.tile_pool(name="ps", bufs=4, space="PSUM") as pspool, \
             tc.tile_pool(name="psb", bufs=4, space="PSUM") as psbpool:
            fA = fpool.tile([128, 64, 10, 32], bf)
            # first w-chunk ahead of the weight stream on sync; the rest on
            # scalar so they don't delay wk[0]
            nc.sync.dma_start(fA[:, 0:8], featA[:, 0:8])
            nc.scalar.dma_start(fA[:, 8:24], featA[:, 8:24])
            nc.scalar.dma_start(fA[:, 24:44], featA[:, 24:44])
            nc.scalar.dma_start(fA[:, 44:64], featA[:, 44:64])
            # zeros for the ACT psum-clearing copies
            zb = fpool.tile([128, 256], bf)
            nc.gpsimd.memset(zb[:], 0.0)
            for hg in range(2):
                hl = 4 * hg
                S = spool.tile([128, 4096], bf)
                for qi in range(4):
                    wk = wkpool.tile([128, 4 * WG_COLS], f8)
                    if hg == 0 and qi == 0:
                        # split the first batch so wg0's weights land sooner
                        nc.sync.dma_start(wk[:, 0:WG_COLS],
                                          wk_d[0][:, 0:WG_COLS])
                        nc.sync.dma_start(wk[:, WG_COLS:4 * WG_COLS],
                                          wk_d[0][:, WG_COLS:4 * WG_COLS])
                    else:
                        nc.sync.dma_start(wk[:], wk_d[hg * 4 + qi])
                    for sub in range(4):
                        wg = 4 * qi + sub
                        w0 = min(4 * wg, 58)  # last group overlaps: w 58..61
                        wr = wk[:, sub * WG_COLS:sub * WG_COLS + WR_COLS]
                        wk64 = wk[:, sub * WG_COLS + WR_COLS:
                                  (sub + 1) * WG_COLS]

                        ps = pspool.tile([128, 256], f32)
                        psB = psbpool.tile([128, 256], f32)
                        # Zero PSUM off the PE (see docstring).
                        nc.scalar.activation(ps[:, :], zb[:, :], ACT_COPY)
                        nc.vector.memset(psB[:, :], 0.0)
                        # taps i in {0,1}: K=128 dual-w stationaries
                        for tau in TAUS:
                            nv, jlo = NV[tau], JLO[tau]
                            for g in range(4):
                                off = TBASE[tau] + g * nv * 64
                                nc.tensor.matmul(
                                    ps[32 * g:32 * g + 32,
                                       64 * jlo:64 * (jlo + nv)],
                                    fA[:, w0 + g, hl + tau, :],
                                    wr[:, off:off + nv * 64],
                                    start=False, stop=False,
                                    skip_group_check=True,
                                    tile_position=(0, 32 * g),
                                )
                        # tap i=2, taus {0,1,4,5}: K=64 lower halves -> main
                        for ti, tau in enumerate(K64_LOW_TAUS):
                            nv, jlo = NV[tau], JLO[tau]
                            for g in range(4):
                                off = K64LO[tau] + g * nv * 64
                                nc.tensor.matmul(
                                    ps[32 * g:32 * g + 32,
                                       64 * jlo:64 * (jlo + nv)],
                                    fA[0:64, w0 + g + 2, hl + tau, :],
                                    wk64[0:64, off:off + nv * 64],
                                    start=False,
                                    stop=(ti == 3 and g == 3),
                                    skip_group_check=True,
                                    tile_position=(0, 32 * g),
                                )
                        # tap i=2, taus {2,3}: K=64 upper halves -> psB
                        for ti, tau in enumerate(K64_HIGH_TAUS):
                            nv, jlo = NV[tau], JLO[tau]
                            for g in range(4):
                                off = K64HI[tau] + g * nv * 64
                                nc.tensor.matmul(
                                    psB[32 * g:32 * g + 32,
                                        64 * jlo:64 * (jlo + nv)],
                                    fA[64:128, w0 + g + 1, hl + tau, :],
                                    wk64[64:128, off:off + nv * 64],
                                    start=False,
                                    stop=(ti == 1 and g == 3),
                                    skip_group_check=True,
                                    tile_position=(64, 32 * g),
                                )
                        sl = S[:, 256 * wg:256 * wg + 256]
                        nc.scalar.activation(sl, psB[:, :], ACT_COPY)
                        nc.vector.tensor_add(sl, ps[:], sl)
                    # quarter-granularity output dump (keeps the tail short)
                    nc.scalar.dma_start(
                        outS[hg][:, 1024 * qi:1024 * (qi + 1)],
                        S[:, 1024 * qi:1024 * (qi + 1)])
    nc.compile()
    return nc


def _get_nc():
    if "nc" not in _STATE:
        _STATE["nc"] = _build_program()
    return _STATE["nc"]


def _quant_w(a):
    return np.clip(a * WSCALE, -15.0, 15.0).astype(F8E3)


def _prep_inputs(features, weights):
    """Build the 8 per-core input dicts (device layouts)."""
    x = np.asarray(features, dtype=np.float32)
    Wt = np.asarray(weights, dtype=np.float32)

    # w-slot -> real w: last group overlaps (w 58..61)
    widx = list(range(60)) + [58, 59, 60, 61]

    in_maps = []
    for s in STARTS:
        xt = x[:, :, s:s + 10, :].transpose(1, 3, 2, 0)    # [c, w, t, b]
        fA = np.zeros((128, 64, 10, 32), dtype=BF16)
        fA[:64] = xt
        fA[64:, 0:63] = xt[:, 1:]                          # w+1 shift

        Wb = Wt[s:s + 8]                                   # [8, 62, o, c, 3, 3]
        Wsel = Wb[:, widx]                                 # [8, 64slots, o, c, 3, 3]
        WT = Wsel.transpose(4, 5, 3, 0, 1, 2)              # [r, i, c, 8h, 64w, o]

        wkf = np.zeros((2, 16, 128, WG_COLS), dtype=np.float32)
        # wr: taps (r, i=d); cols per (tau, g): q -> j=jlo+q, r=tau-j
        wr = wkf[:, :, :, 0:WR_COLS]
        for tau in TAUS:
            nv, jlo = NV[tau], JLO[tau]
            view = wr[:, :, :, TBASE[tau]:TBASE[tau + 1]].reshape(
                2, 16, 128, 4, nv, 64)
            for q in range(nv):
                j = jlo + q
                r = tau - j
                for d in range(2):
                    src = WT[r, d].reshape(CIN, 2, 4, 16, 4, COUT)[:, :, j]
                    view[:, :, d * 64:(d + 1) * 64, :, q, :] = \
                        src.transpose(1, 2, 0, 3, 4)       # [hg, wg, c, g, o]
        # wk64: tap i=2; taus {0,1,4,5} at partitions 0:64, {2,3} at 64:128
        wk64 = wkf[:, :, :, WR_COLS:WG_COLS]
        for tau in TAUS:
            nv, jlo = NV[tau], JLO[tau]
            if tau in K64LO:
                p0, cb = 0, K64LO[tau]
            else:
                p0, cb = 64, K64HI[tau]
            view = wk64[:, :, p0:p0 + 64, cb:cb + 4 * nv * 64].reshape(
                2, 16, 64, 4, nv, 64)
            for q in range(nv):
                j = jlo + q
                r = tau - j
                src = WT[r, 2].reshape(CIN, 2, 4, 16, 4, COUT)[:, :, j]
                view[:, :, :, :, q, :] = src.transpose(1, 2, 0, 3, 4)
        # [2, 16, 128, 4608] -> [8(hg*4+qi), 128, 18432]
        wk = _quant_w(wkf).reshape(2, 4, 4, 128, WG_COLS).transpose(
            0, 1, 3, 2, 4).reshape(8, 128, 4 * WG_COLS)
        wk = np.ascontiguousarray(wk)
        in_maps.append({"featA": fA, "wk": wk})
    return in_maps


def _gather(results, bias):
    out = np.zeros((B, COUT, HOUT, WOUT), dtype=np.float32)
    inv = 1.0 / float(WSCALE)
    for core, s in enumerate(STARTS):
        arr = np.asarray(results[core]["outS"]).astype(np.float32) * inv
        # [hg, g, b, wg, j, o] -> [b, o, hg, j, wg, g]
        arr = arr.reshape(2, 4, 32, 16, 4, 64).transpose(2, 5, 0, 4, 3, 1)
        arr = arr.reshape(32, 64, 8, 64)
        out[:, :, s:s + 8, 0:60] = arr[:, :, :, 0:60]
        out[:, :, s:s + 8, 60:62] = arr[:, :, :, 62:64]
    out += np.asarray(bias, dtype=np.float32).transpose(2, 0, 1)[None]
    return out


def _run(in_maps, trace=False, trace_cores=None):
    from concourse.bass_utils import run_bass_kernel_spmd
    nc = _get_nc()
    return run_bass_kernel_spmd(
        nc, in_maps, core_ids=list(range(NCORES)),
        trace=trace, trace_cores=trace_cores,
    )


def kernel(features, weights, bias):
    in_maps = _prep_inputs(features, weights)
    res = _run(in_maps)
    return _gather(res.results, bias)


# revision 12
# speedup vs baseline: 1.1522x; 1.0007x over previous
"""LocallyConnected2d kernel for 8 TRN2 NeuronCores (Bass/Tile).

Problem (hardcoded):
  features [32, 64, 64, 64] f32, weights [62, 62, 64, 64, 3, 3] f32,
  bias [62, 62, 64] f32 -> out [32, 64, 62, 62] f32
  out[b,o,h,w] = sum_{c,i,j} x[b,c,h+i,w+j] * W[h,w,o,c,i,j] + bias[h,w,o]

Strategy (v5):
  - Shard over Hout: 8 cores x 8 output rows (bands [0,8,...,48,54], last two
    overlap; host takes canonical rows from each core).
  - Weights stream as fp8 e3m4 (x2 scale, /2 on host) = 1 B/el -> 18.9 MB/core
    with zero padding waste; activations stay bf16 (mixed-dtype matmul: only
    fp32 operands must be paired). PSUM accumulates fp32. rel err ~0.014.
  - fA layout [128=(c | c shifted w+1), w, t, b]. Only the lower half (2.6 MB)
    comes from HBM; the w+1-shifted upper half is derived on-chip with
    SBUF->SBUF DMAs on the (idle) gpsimd SWDGE path, chunked to follow the
    HBM chunks. fA[64:,63] is never read, so it needs no init.
  - Per (hg=4-row half-band, wg=4-w group), outputs in PSUM [128,256]:
    partitions=(4w x 32b) via col tile_position, free=(4j x 64 cout).
    * PSUM is zeroed OFF the PE (ACT copy-from-zeros / DVE memset); all
      matmuls run flags=0 (has_written: accumulate where stale-set onto the
      zeros, overwrite the zeros where clear - correct either way). No K=1
      zeroing matmuls, no start=True anywhere.
    * wr MMs (taps i in {0,1}): stationary fA[:, w0+g, hl+tau] K=128, moving
      N=nv*64, tau=j+r grouped. 24 MMs per (hg,wg).
    * tap i=2 is K=64, tau-grouped, split across TWO psum tiles because one
      accumulation group must not mix PE row-tiles (HW limitation, probed):
      taus {0,1,4,5} via fA lower -> main ps (row tile 0); taus {2,3} via
      the w+1-shifted upper half (same x column) -> psB (row tile 64).
      Both partition halves of wk64 carry real data -> no padding bytes.
    * combine: ACT copies psB -> S slice (bf16), DVE tensor_add(S, ps, S).
  - DMA: fA w-chunk [0:8] first on sync, then 8 wk batch-transfers
    ([128,18432] fp8 = 2.36 MB, first one split so wg0 starts early); fA
    chunks 2-4 + quarter-granularity outS dumps ride the scalar ring.
"""

import numpy as np
import ml_dtypes

BF16 = ml_dtypes.bfloat16
F8E3 = ml_dtypes.float8_e3m4
WSCALE = np.float32(2.0)

B, CIN, COUT = 32, 64, 64
H = W = 64
HOUT = WOUT = 62
NCORES = 8
STARTS = [0, 8, 16, 24, 32, 40, 48, 54]

# tau-group geometry: tau = t - hl in 0..5; valid out-rows j in [jlo, jhi]
TAUS = list(range(6))
JLO = [max(0, t - 2) for t in TAUS]
JHI = [min(3, t) for t in TAUS]
NV = [hi - lo + 1 for lo, hi in zip(JLO, JHI)]            # [1,2,3,3,2,1]
TBASE = [0]
for t in TAUS:
    TBASE.append(TBASE[-1] + 4 * NV[t] * 64)              # wr per-tau base col
WR_COLS = TBASE[-1]                                        # 3072
# K64 split: taus 0,1,4,5 -> lower half (row tile 0, main psum);
#            taus 2,3     -> upper half (row tile 64, psB)
K64_LOW_TAUS = [0, 1, 4, 5]
K64_HIGH_TAUS = [2, 3]
K64LO = {}
off = 0
for t in K64_LOW_TAUS:
    K64LO[t] = off
    off += 4 * NV[t] * 64
K64HI = {}
off = 0
for t in K64_HIGH_TAUS:
    K64HI[t] = off
    off += 4 * NV[t] * 64
K64_COLS = 1536                                            # both halves
WG_COLS = WR_COLS + K64_COLS                               # 4608

_STATE = {}


def _build_program():
    import concourse.tile as tile
    from concourse import bacc, mybir

    bf = mybir.dt.bfloat16
    f8 = mybir.dt.float8e3
    f32 = mybir.dt.float32
    ACT_COPY = mybir.ActivationFunctionType.Copy

    nc = bacc.Bacc(None, target_bir_lowering=False)
    featA = nc.dram_tensor("featA", [128, 64, 10, 32], bf, kind="ExternalInput")
    wk_d = nc.dram_tensor("wk", [16, 128, 2 * WG_COLS], f8, kind="ExternalInput")
    outS = nc.dram_tensor("outS", [2, 128, 4096], bf, kind="ExternalOutput")

    with tile.TileContext(nc) as tc:
        with tc.tile_pool(name="feat", bufs=1) as fpool, \
             tc.tile_pool(name="wk", bufs=4) as wkpool, \
             tc.tile_pool(name="st", bufs=2) as spool, \
             tc.tile_pool(name="ps", bufs=4, space="PSUM") as pspool, \
             tc.tile_pool(name="psb", bufs=4, space="PSUM") as psbpool:
            fA = fpool.tile([128, 64, 10, 32], bf)
            # first w-chunk ahead of the weight stream on sync; the rest on
            # scalar so they don't delay wk[0]
            nc.scalar.dma_start(fA[:, 0:8], featA[:, 0:8])
            nc.scalar.dma_start(fA[:, 8:24], featA[:, 8:24])
            nc.scalar.dma_start(fA[:, 24:44], featA[:, 24:44])
            nc.scalar.dma_start(fA[:, 44:64], featA[:, 44:64])
            # zeros for the ACT psum-clearing copies
            zb = fpool.tile([128, 256], bf)
            nc.gpsimd.memset(zb[:], 0.0)
            for hg in range(2):
                hl = 4 * hg
                S = spool.tile([128, 4096], bf)
                for pi in range(8):
                    wk = wkpool.tile([128, 2 * WG_COLS], f8)
                    if hg == 0 and pi == 0:
                        # split the first pair so wg0's weights land sooner
                        nc.sync.dma_start(wk[:, 0:WG_COLS],
                                          wk_d[0][:, 0:WG_COLS])
                        nc.sync.dma_start(wk[:, WG_COLS:2 * WG_COLS],
                                          wk_d[0][:, WG_COLS:2 * WG_COLS])
                    else:
                        nc.sync.dma_start(wk[:], wk_d[hg * 8 + pi])
                    for sub in range(2):
                        wg = 2 * pi + sub
                        w0 = min(4 * wg, 58)  # last group overlaps: w 58..61
                        wr = wk[:, sub * WG_COLS:sub * WG_COLS + WR_COLS]
                        wk64 = wk[:, sub * WG_COLS + WR_COLS:
                                  (sub + 1) * WG_COLS]

                        ps = pspool.tile([128, 256], f32)
                        psB = psbpool.tile([128, 256], f32)
                        # Zero PSUM off the PE (see docstring).
                        nc.scalar.activation(ps[:, :], zb[:, :], ACT_COPY)
                        nc.vector.memset(psB[:, :], 0.0)
                        # taps i in {0,1}: K=128 dual-w stationaries
                        for tau in TAUS:
                            nv, jlo = NV[tau], JLO[tau]
                            for g in range(4):
                                off = TBASE[tau] + g * nv * 64
                                nc.tensor.matmul(
                                    ps[32 * g:32 * g + 32,
                                       64 * jlo:64 * (jlo + nv)],
                                    fA[:, w0 + g, hl + tau, :],
                                    wr[:, off:off + nv * 64],
                                    start=False, stop=False,
                                    skip_group_check=True,
                                    tile_position=(0, 32 * g),
                                )
                        # tap i=2, taus {2,3}: K=64 upper halves -> psB
                        for ti, tau in enumerate(K64_HIGH_TAUS):
                            nv, jlo = NV[tau], JLO[tau]
                            for g in range(4):
                                off = K64HI[tau] + g * nv * 64
                                nc.tensor.matmul(
                                    psB[32 * g:32 * g + 32,
                                        64 * jlo:64 * (jlo + nv)],
                                    fA[64:128, w0 + g + 1, hl + tau, :],
                                    wk64[64:128, off:off + nv * 64],
                                    start=False,
                                    stop=(ti == 1 and g == 3),
                                    skip_group_check=True,
                                    tile_position=(64, 32 * g),
                                )
                        # tap i=2, taus {0,1,4,5}: K=64 lower halves -> main
                        for ti, tau in enumerate(K64_LOW_TAUS):
                            nv, jlo = NV[tau], JLO[tau]
                            for g in range(4):
                                off = K64LO[tau] + g * nv * 64
                                nc.tensor.matmul(
                                    ps[32 * g:32 * g + 32,
                                       64 * jlo:64 * (jlo + nv)],
                                    fA[0:64, w0 + g + 2, hl + tau, :],
                                    wk64[0:64, off:off + nv * 64],
                                    start=False,
                                    stop=(ti == 3 and g == 3),
                                    skip_group_check=True,
                                    tile_position=(0, 32 * g),
                                )
                        sl = S[:, 256 * wg:256 * wg + 256]
                        nc.scalar.activation(sl, psB[:, :], ACT_COPY)
                        nc.vector.tensor_add(sl, ps[:], sl)
                    if pi % 2 == 1:
                        # quarter-granularity dump (keeps the tail short)
                        qq = pi // 2
                        nc.scalar.dma_start(
                            outS[hg][:, 1024 * qq:1024 * (qq + 1)],
                            S[:, 1024 * qq:1024 * (qq + 1)])
    nc.compile()
    return nc


def _get_nc():
    if "nc" not in _STATE:
        _STATE["nc"] = _build_program()
    return _STATE["nc"]


def _quant_w(a):
    return np.clip(a * WSCALE, -15.0, 15.0).astype(F8E3)


def _prep_inputs(features, weights):
    """Build the 8 per-core input dicts (device layouts)."""
    x = np.asarray(features, dtype=np.float32)
    Wt = np.asarray(weights, dtype=np.float32)

    # w-slot -> real w: last group overlaps (w 58..61)
    widx = list(range(60)) + [58, 59, 60, 61]

    in_maps = []
    for s in STARTS:
        xt = x[:, :, s:s + 10, :].transpose(1, 3, 2, 0)    # [c, w, t, b]
        fA = np.zeros((128, 64, 10, 32), dtype=BF16)
        fA[:64] = xt
        fA[64:, 0:63] = xt[:, 1:]                          # w+1 shift

        Wb = Wt[s:s + 8]                                   # [8, 62, o, c, 3, 3]
        Wsel = Wb[:, widx]                                 # [8, 64slots, o, c, 3, 3]
        WT = Wsel.transpose(4, 5, 3, 0, 1, 2)              # [r, i, c, 8h, 64w, o]

        wkf = np.zeros((2, 16, 128, WG_COLS), dtype=np.float32)
        # wr: taps (r, i=d); cols per (tau, g): q -> j=jlo+q, r=tau-j
        wr = wkf[:, :, :, 0:WR_COLS]
        for tau in TAUS:
            nv, jlo = NV[tau], JLO[tau]
            view = wr[:, :, :, TBASE[tau]:TBASE[tau + 1]].reshape(
                2, 16, 128, 4, nv, 64)
            for q in range(nv):
                j = jlo + q
                r = tau - j
                for d in range(2):
                    src = WT[r, d].reshape(CIN, 2, 4, 16, 4, COUT)[:, :, j]
                    view[:, :, d * 64:(d + 1) * 64, :, q, :] = \
                        src.transpose(1, 2, 0, 3, 4)       # [hg, wg, c, g, o]
        # wk64: tap i=2; taus {0,1,4,5} at partitions 0:64, {2,3} at 64:128
        wk64 = wkf[:, :, :, WR_COLS:WG_COLS]
        for tau in TAUS:
            nv, jlo = NV[tau], JLO[tau]
            if tau in K64LO:
                p0, cb = 0, K64LO[tau]
            else:
                p0, cb = 64, K64HI[tau]
            view = wk64[:, :, p0:p0 + 64, cb:cb + 4 * nv * 64].reshape(
                2, 16, 64, 4, nv, 64)
            for q in range(nv):
                j = jlo + q
                r = tau - j
                src = WT[r, 2].reshape(CIN, 2, 4, 16, 4, COUT)[:, :, j]
                view[:, :, :, :, q, :] = src.transpose(1, 2, 0, 3, 4)
        # [2, 16, 128, 4608] -> [16(hg*8+pi), 128, 9216]
        wk = _quant_w(wkf).reshape(2, 8, 2, 128, WG_COLS).transpose(
            0, 1, 3, 2, 4).reshape(16, 128, 2 * WG_COLS)
        wk = np.ascontiguousarray(wk)
        in_maps.append({"featA": fA, "wk": wk})
    return in_maps


def _gather(results, bias):
    out = np.zeros((B, COUT, HOUT, WOUT), dtype=np.float32)
    inv = 1.0 / float(WSCALE)
    for core, s in enumerate(STARTS):
        arr = np.asarray(results[core]["outS"]).astype(np.float32) * inv
        # [hg, g, b, wg, j, o] -> [b, o, hg, j, wg, g]
        arr = arr.reshape(2, 4, 32, 16, 4, 64).transpose(2, 5, 0, 4, 3, 1)
        arr = arr.reshape(32, 64, 8, 64)
        out[:, :, s:s + 8, 0:60] = arr[:, :, :, 0:60]
        out[:, :, s:s + 8, 60:62] = arr[:, :, :, 62:64]
    out += np.asarray(bias, dtype=np.float32).transpose(2, 0, 1)[None]
    return out


def _run(in_maps, trace=False, trace_cores=None):
    from concourse.bass_utils import run_bass_kernel_spmd
    nc = _get_nc()
    return run_bass_kernel_spmd(
        nc, in_maps, core_ids=list(range(NCORES)),
        trace=trace, trace_cores=trace_cores,
    )


def kernel(features, weights, bias):
    in_maps = _prep_inputs(features, weights)
    res = _run(in_maps)
    return _gather(res.results, bias)


# revision 13
# speedup vs baseline: 1.1795x; 1.0236x over previous
"""LocallyConnected2d kernel for 8 TRN2 NeuronCores (Bass/Tile).

Problem (hardcoded):
  features [32, 64, 64, 64] f32, weights [62, 62, 64, 64, 3, 3] f32,
  bias [62, 62, 64] f32 -> out [32, 64, 62, 62] f32
  out[b,o,h,w] = sum_{c,i,j} x[b,c,h+i,w+j] * W[h,w,o,c,i,j] + bias[h,w,o]

Strategy (v5):
  - Shard over Hout: 8 cores x 8 output rows (bands [0,8,...,48,54], last two
    overlap; host takes canonical rows from each core).
  - Weights stream as fp8 e3m4 (x2 scale, /2 on host) = 1 B/el -> 18.9 MB/core
    with zero padding waste; activations stay bf16 (mixed-dtype matmul: only
    fp32 operands must be paired). PSUM accumulates fp32. rel err ~0.014.
  - fA layout [128=(c | c shifted w+1), w, t, b]. Only the lower half (2.6 MB)
    comes from HBM; the w+1-shifted upper half is derived on-chip with
    SBUF->SBUF DMAs on the (idle) gpsimd SWDGE path, chunked to follow the
    HBM chunks. fA[64:,63] is never read, so it needs no init.
  - Per (hg=4-row half-band, wg=4-w group), outputs in PSUM [128,256]:
    partitions=(4w x 32b) via col tile_position, free=(4j x 64 cout).
    * PSUM is zeroed OFF the PE (ACT copy-from-zeros / DVE memset); all
      matmuls run flags=0 (has_written: accumulate where stale-set onto the
      zeros, overwrite the zeros where clear - correct either way). No K=1
      zeroing matmuls, no start=True anywhere.
    * wr MMs (taps i in {0,1}): stationary fA[:, w0+g, hl+tau] K=128, moving
      N=nv*64, tau=j+r grouped. 24 MMs per (hg,wg).
    * tap i=2 is K=64, tau-grouped, split across TWO psum tiles because one
      accumulation group must not mix PE row-tiles (HW limitation, probed):
      taus {0,1,4,5} via fA lower -> main ps (row tile 0); taus {2,3} via
      the w+1-shifted upper half (same x column) -> psB (row tile 64).
      Both partition halves of wk64 carry real data -> no padding bytes.
    * combine: ACT copies psB -> S slice (bf16), DVE tensor_add(S, ps, S).
  - DMA: fA w-chunk [0:8] first on sync, then 8 wk batch-transfers
    ([128,18432] fp8 = 2.36 MB, first one split so wg0 starts early); fA
    chunks 2-4 + quarter-granularity outS dumps ride the scalar ring.
"""

import numpy as np
import ml_dtypes

BF16 = ml_dtypes.bfloat16
F8E3 = ml_dtypes.float8_e3m4
WSCALE = np.float32(2.0)

B, CIN, COUT = 32, 64, 64
H = W = 64
HOUT = WOUT = 62
NCORES = 8
STARTS = [0, 8, 16, 24, 32, 40, 48, 54]

# tau-group geometry: tau = t - hl in 0..5; valid out-rows j in [jlo, jhi]
TAUS = list(range(6))
JLO = [max(0, t - 2) for t in TAUS]
JHI = [min(3, t) for t in TAUS]
NV = [hi - lo + 1 for lo, hi in zip(JLO, JHI)]            # [1,2,3,3,2,1]
TBASE = [0]
for t in TAUS:
    TBASE.append(TBASE[-1] + 4 * NV[t] * 64)              # wr per-tau base col
WR_COLS = TBASE[-1]                                        # 3072
# K64 split: taus 0,1,4,5 -> lower half (row tile 0, main psum);
#            taus 2,3     -> upper half (row tile 64, psB)
K64_LOW_TAUS = [0, 1, 4, 5]
K64_HIGH_TAUS = [2, 3]
K64LO = {}
off = 0
for t in K64_LOW_TAUS:
    K64LO[t] = off
    off += 4 * NV[t] * 64
K64HI = {}
off = 0
for t in K64_HIGH_TAUS:
    K64HI[t] = off
    off += 4 * NV[t] * 64
K64_COLS = 1536                                            # both halves
WG_COLS = WR_COLS + K64_COLS                               # 4608

_STATE = {}


def _build_program():
    import concourse.tile as tile
    from concourse import bacc, mybir

    bf = mybir.dt.bfloat16
    f8 = mybir.dt.float8e3
    f32 = mybir.dt.float32
    ACT_COPY = mybir.ActivationFunctionType.Copy

    nc = bacc.Bacc(None, target_bir_lowering=False)
    featA = nc.dram_tensor("featA", [128, 64, 10, 32], bf, kind="ExternalInput")
    wk_d = nc.dram_tensor("wk", [32, 128, WG_COLS], f8, kind="ExternalInput")
    outS = nc.dram_tensor("outS", [2, 128, 4096], bf, kind="ExternalOutput")

    with tile.TileContext(nc) as tc:
        with tc.tile_pool(name="feat", bufs=1) as fpool, \
             tc.tile_pool(name="wk", bufs=6) as wkpool, \
             tc.tile_pool(name="st", bufs=2) as spool, \
             tc.tile_pool(name="ps", bufs=4, space="PSUM") as pspool, \
             tc.tile_pool(name="psb", bufs=4, space="PSUM") as psbpool:
            fA = fpool.tile([128, 64, 10, 32], bf)
            # first w-chunk ahead of the weight stream on sync; the rest on
            # scalar so they don't delay wk[0]
            nc.scalar.dma_start(fA[:, 0:8], featA[:, 0:8])
            nc.scalar.dma_start(fA[:, 8:24], featA[:, 8:24])
            nc.scalar.dma_start(fA[:, 24:44], featA[:, 24:44])
            nc.scalar.dma_start(fA[:, 44:64], featA[:, 44:64])
            # zeros for the ACT psum-clearing copies
            zb = fpool.tile([128, 256], bf)
            nc.gpsimd.memset(zb[:], 0.0)
            for hg in range(2):
                hl = 4 * hg
                S = spool.tile([128, 4096], bf)
                for wg in range(16):
                    wk = wkpool.tile([128, WG_COLS], f8)
                    nc.sync.dma_start(wk[:], wk_d[hg * 16 + wg])
                    if True:
                        w0 = min(4 * wg, 58)  # last group overlaps: w 58..61
                        wr = wk[:, 0:WR_COLS]
                        wk64 = wk[:, WR_COLS:WG_COLS]

                        ps = pspool.tile([128, 256], f32)
                        psB = psbpool.tile([128, 256], f32)
                        # Zero PSUM off the PE (see docstring).
                        nc.scalar.activation(ps[:, :], zb[:, :], ACT_COPY)
                        nc.vector.memset(psB[:, :], 0.0)
                        # taps i in {0,1}: K=128 dual-w stationaries
                        for tau in TAUS:
                            nv, jlo = NV[tau], JLO[tau]
                            for g in range(4):
                                off = TBASE[tau] + g * nv * 64
                                nc.tensor.matmul(
                                    ps[32 * g:32 * g + 32,
                                       64 * jlo:64 * (jlo + nv)],
                                    fA[:, w0 + g, hl + tau, :],
                                    wr[:, off:off + nv * 64],
                                    start=False, stop=False,
                                    skip_group_check=True,
                                    tile_position=(0, 32 * g),
                                )
                        # tap i=2, taus {2,3}: K=64 upper halves -> psB
                        for ti, tau in enumerate(K64_HIGH_TAUS):
                            nv, jlo = NV[tau], JLO[tau]
                            for g in range(4):
                                off = K64HI[tau] + g * nv * 64
                                nc.tensor.matmul(
                                    psB[32 * g:32 * g + 32,
                                        64 * jlo:64 * (jlo + nv)],
                                    fA[64:128, w0 + g + 1, hl + tau, :],
                                    wk64[64:128, off:off + nv * 64],
                                    start=False,
                                    stop=(ti == 1 and g == 3),
                                    skip_group_check=True,
                                    tile_position=(64, 32 * g),
                                )
                        # tap i=2, taus {0,1,4,5}: K=64 lower halves -> main
                        for ti, tau in enumerate(K64_LOW_TAUS):
                            nv, jlo = NV[tau], JLO[tau]
                            for g in range(4):
                                off = K64LO[tau] + g * nv * 64
                                nc.tensor.matmul(
                                    ps[32 * g:32 * g + 32,
                                       64 * jlo:64 * (jlo + nv)],
                                    fA[0:64, w0 + g + 2, hl + tau, :],
                                    wk64[0:64, off:off + nv * 64],
                                    start=False,
                                    stop=(ti == 3 and g == 3),
                                    skip_group_check=True,
                                    tile_position=(0, 32 * g),
                                )
                        sl = S[:, 256 * wg:256 * wg + 256]
                        nc.scalar.activation(sl, psB[:, :], ACT_COPY)
                        nc.vector.tensor_add(sl, ps[:], sl)
                    if wg % 4 == 3:
                        # quarter-granularity dump (keeps the tail short)
                        qq = wg // 4
                        nc.scalar.dma_start(
                            outS[hg][:, 1024 * qq:1024 * (qq + 1)],
                            S[:, 1024 * qq:1024 * (qq + 1)])
    nc.compile()
    return nc


def _get_nc():
    if "nc" not in _STATE:
        _STATE["nc"] = _build_program()
    return _STATE["nc"]


def _quant_w(a):
    return np.clip(a * WSCALE, -15.0, 15.0).astype(F8E3)


def _prep_inputs(features, weights):
    """Build the 8 per-core input dicts (device layouts)."""
    x = np.asarray(features, dtype=np.float32)
    Wt = np.asarray(weights, dtype=np.float32)

    # w-slot -> real w: last group overlaps (w 58..61)
    widx = list(range(60)) + [58, 59, 60, 61]

    in_maps = []
    for s in STARTS:
        xt = x[:, :, s:s + 10, :].transpose(1, 3, 2, 0)    # [c, w, t, b]
        fA = np.zeros((128, 64, 10, 32), dtype=BF16)
        fA[:64] = xt
        fA[64:, 0:63] = xt[:, 1:]                          # w+1 shift

        Wb = Wt[s:s + 8]                                   # [8, 62, o, c, 3, 3]
        Wsel = Wb[:, widx]                                 # [8, 64slots, o, c, 3, 3]
        WT = Wsel.transpose(4, 5, 3, 0, 1, 2)              # [r, i, c, 8h, 64w, o]

        wkf = np.zeros((2, 16, 128, WG_COLS), dtype=np.float32)
        # wr: taps (r, i=d); cols per (tau, g): q -> j=jlo+q, r=tau-j
        wr = wkf[:, :, :, 0:WR_COLS]
        for tau in TAUS:
            nv, jlo = NV[tau], JLO[tau]
            view = wr[:, :, :, TBASE[tau]:TBASE[tau + 1]].reshape(
                2, 16, 128, 4, nv, 64)
            for q in range(nv):
                j = jlo + q
                r = tau - j
                for d in range(2):
                    src = WT[r, d].reshape(CIN, 2, 4, 16, 4, COUT)[:, :, j]
                    view[:, :, d * 64:(d + 1) * 64, :, q, :] = \
                        src.transpose(1, 2, 0, 3, 4)       # [hg, wg, c, g, o]
        # wk64: tap i=2; taus {0,1,4,5} at partitions 0:64, {2,3} at 64:128
        wk64 = wkf[:, :, :, WR_COLS:WG_COLS]
        for tau in TAUS:
            nv, jlo = NV[tau], JLO[tau]
            if tau in K64LO:
                p0, cb = 0, K64LO[tau]
            else:
                p0, cb = 64, K64HI[tau]
            view = wk64[:, :, p0:p0 + 64, cb:cb + 4 * nv * 64].reshape(
                2, 16, 64, 4, nv, 64)
            for q in range(nv):
                j = jlo + q
                r = tau - j
                src = WT[r, 2].reshape(CIN, 2, 4, 16, 4, COUT)[:, :, j]
                view[:, :, :, :, q, :] = src.transpose(1, 2, 0, 3, 4)
        # [2, 16, 128, 4608] -> [32(hg*16+wg), 128, 4608]
        wk = _quant_w(wkf).reshape(32, 128, WG_COLS)
        wk = np.ascontiguousarray(wk)
        in_maps.append({"featA": fA, "wk": wk})
    return in_maps


def _gather(results, bias):
    out = np.zeros((B, COUT, HOUT, WOUT), dtype=np.float32)
    inv = 1.0 / float(WSCALE)
    for core, s in enumerate(STARTS):
        arr = np.asarray(results[core]["outS"]).astype(np.float32) * inv
        # [hg, g, b, wg, j, o] -> [b, o, hg, j, wg, g]
        arr = arr.reshape(2, 4, 32, 16, 4, 64).transpose(2, 5, 0, 4, 3, 1)
        arr = arr.reshape(32, 64, 8, 64)
        out[:, :, s:s + 8, 0:60] = arr[:, :, :, 0:60]
        out[:, :, s:s + 8, 60:62] = arr[:, :, :, 62:64]
    out += np.asarray(bias, dtype=np.float32).transpose(2, 0, 1)[None]
    return out


def _run(in_maps, trace=False, trace_cores=None):
    from concourse.bass_utils import run_bass_kernel_spmd
    nc = _get_nc()
    return run_bass_kernel_spmd(
        nc, in_maps, core_ids=list(range(NCORES)),
        trace=trace, trace_cores=trace_cores,
    )


def kernel(features, weights, bias):
    in_maps = _prep_inputs(features, weights)
    res = _run(in_maps)
    return _gather(res.results, bias)


# revision 14
# speedup vs baseline: 1.2610x; 1.0691x over previous
"""LocallyConnected2d kernel for 8 TRN2 NeuronCores (Bass/Tile).

Problem (hardcoded):
  features [32, 64, 64, 64] f32, weights [62, 62, 64, 64, 3, 3] f32,
  bias [62, 62, 64] f32 -> out [32, 64, 62, 62] f32
  out[b,o,h,w] = sum_{c,i,j} x[b,c,h+i,w+j] * W[h,w,o,c,i,j] + bias[h,w,o]

Strategy (v9 - full 8-row band per work unit):
  - Shard over Hout: 8 cores x 8 output rows (bands [0,8,...,48,54], last two
    overlap; host takes canonical rows from each core).
  - Weights stream as fp8 e3m4 (x2 scale, /2 on host) = 1 B/el -> 18.9 MB/core
    with zero padding waste; activations stay bf16 (mixed-dtype matmul: only
    fp32 operands must be paired). PSUM accumulates fp32. rel err ~0.014.
  - fA layout [128=(c | c shifted w+1), w, t, b]; a [128,32] slice at (w,t) is
    an im2col patch: lower half = x(w), upper = x(w+1).
  - Work unit = one wg (4 w positions), ALL 8 output rows at once: PSUM
    [128, 512] (one full bank): partitions=(4w x 32b) via col tile_position,
    free=(8j x 64 cout). tau = t in 0..9, valid j in [max(0,t-2), min(7,t)].
    * PSUM zeroed OFF the PE (ACT copy-from-zeros / DVE memset); matmuls all
      run flags=0 (has_written: accumulate onto the zeros where stale-set,
      overwrite the zeros where clear). No start=True anywhere.
    * wr MMs (taps i in {0,1}): stationary fA[:, w0+g, tau] K=128, moving
      N=nv*64 <= 192. 40 MMs/wg.
    * tap i=2 is K=64, tau-grouped, split across TWO psum tiles because one
      accumulation group must not mix PE row-tiles (HW limitation, probed):
      taus {0,1,2,5,8,9} via fA lower (w0+g+2) -> main ps (row tile 0);
      taus {3,4,6,7} via the w+1-shifted upper half (w0+g+1, same x column)
      -> psB (row tile 64). psB's j0 columns are never written and stay
      memset-zero. Both wk64 partition halves fully used -> no padding.
    * combine: ACT copies psB -> S slice (bf16), DVE tensor_add(S, ps, S).
  - DMA: 16 wk transfers [128, 9216] fp8 (1.18 MB) on sync; fA w-chunks +
    eighth-granularity outS dumps on scalar.
"""

import numpy as np
import ml_dtypes

BF16 = ml_dtypes.bfloat16
F8E3 = ml_dtypes.float8_e3m4
WSCALE = np.float32(2.0)

B, CIN, COUT = 32, 64, 64
H = W = 64
HOUT = WOUT = 62
NCORES = 8
STARTS = [0, 8, 16, 24, 32, 40, 48, 54]

# tau-group geometry over the full 8-row band: tau = t in 0..9
TAUS = list(range(10))
JLO = [max(0, t - 2) for t in TAUS]
JHI = [min(7, t) for t in TAUS]
NV = [hi - lo + 1 for lo, hi in zip(JLO, JHI)]    # [1,2,3,3,3,3,3,3,2,1]
TBASE = [0]
for t in TAUS:
    TBASE.append(TBASE[-1] + 4 * NV[t] * 64)
WR_COLS = TBASE[-1]                                # 6144
# K64 split: 12 nv-units each side
K64_LOW_TAUS = [0, 1, 2, 5, 8, 9]                  # nv 1+2+3+3+2+1 = 12
K64_HIGH_TAUS = [3, 4, 6, 7]                       # nv 3+3+3+3 = 12
K64LO = {}
off = 0
for t in K64_LOW_TAUS:
    K64LO[t] = off
    off += 4 * NV[t] * 64
K64HI = {}
off = 0
for t in K64_HIGH_TAUS:
    K64HI[t] = off
    off += 4 * NV[t] * 64
K64_COLS = 3072                                    # both halves, 12 units
WG_COLS = WR_COLS + K64_COLS                       # 9216

_STATE = {}


def _build_program():
    import concourse.tile as tile
    from concourse import bacc, mybir

    bf = mybir.dt.bfloat16
    f8 = mybir.dt.float8e3
    f32 = mybir.dt.float32
    ACT_COPY = mybir.ActivationFunctionType.Copy

    nc = bacc.Bacc(None, target_bir_lowering=False)
    featA = nc.dram_tensor("featA", [128, 64, 10, 32], bf, kind="ExternalInput")
    wk_d = nc.dram_tensor("wk", [16, 128, WG_COLS], f8, kind="ExternalInput")
    outS = nc.dram_tensor("outS", [128, 8192], bf, kind="ExternalOutput")

    with tile.TileContext(nc) as tc:
        with tc.tile_pool(name="feat", bufs=1) as fpool, \
             tc.tile_pool(name="wk", bufs=5) as wkpool, \
             tc.tile_pool(name="st", bufs=1) as spool, \
             tc.tile_pool(name="ps", bufs=4, space="PSUM") as pspool, \
             tc.tile_pool(name="psb", bufs=4, space="PSUM") as psbpool:
            fA = fpool.tile([128, 64, 10, 32], bf)
            nc.scalar.dma_start(fA[:, 0:8], featA[:, 0:8])
            nc.scalar.dma_start(fA[:, 8:24], featA[:, 8:24])
            nc.scalar.dma_start(fA[:, 24:44], featA[:, 24:44])
            nc.scalar.dma_start(fA[:, 44:64], featA[:, 44:64])
            # zeros for the ACT psum-clearing copies
            zb = fpool.tile([128, 512], bf)
            nc.gpsimd.memset(zb[:], 0.0)
            S = spool.tile([128, 8192], bf)
            for wg in range(16):
                wk = wkpool.tile([128, WG_COLS], f8)
                nc.sync.dma_start(wk[:], wk_d[wg])
                w0 = min(4 * wg, 58)      # last group overlaps: w 58..61
                wr = wk[:, 0:WR_COLS]
                wk64 = wk[:, WR_COLS:WG_COLS]

                ps = pspool.tile([128, 512], f32)
                psB = psbpool.tile([128, 512], f32)
                # Zero PSUM off the PE (see docstring).
                nc.scalar.activation(ps[:, :], zb[:, :], ACT_COPY)
                nc.vector.memset(psB[:, :], 0.0)
                # tap i=2, taus {3,4,6,7}: K=64 upper halves -> psB first
                # (psB finishes early so its ACT copy overlaps later MMs)
                for ti, tau in enumerate(K64_HIGH_TAUS):
                    nv, jlo = NV[tau], JLO[tau]
                    for g in range(4):
                        off = K64HI[tau] + g * nv * 64
                        nc.tensor.matmul(
                            psB[32 * g:32 * g + 32,
                                64 * jlo:64 * (jlo + nv)],
                            fA[64:128, w0 + g + 1, tau, :],
                            wk64[64:128, off:off + nv * 64],
                            start=False,
                            stop=(ti == 3 and g == 3),
                            skip_group_check=True,
                            tile_position=(64, 32 * g),
                        )
                # taps i in {0,1}: K=128 dual-w stationaries
                for tau in TAUS:
                    nv, jlo = NV[tau], JLO[tau]
                    for g in range(4):
                        off = TBASE[tau] + g * nv * 64
                        nc.tensor.matmul(
                            ps[32 * g:32 * g + 32,
                               64 * jlo:64 * (jlo + nv)],
                            fA[:, w0 + g, tau, :],
                            wr[:, off:off + nv * 64],
                            start=False, stop=False,
                            skip_group_check=True,
                            tile_position=(0, 32 * g),
                        )
                # tap i=2, taus {0,1,2,5,8,9}: K=64 lower halves -> main
                for ti, tau in enumerate(K64_LOW_TAUS):
                    nv, jlo = NV[tau], JLO[tau]
                    for g in range(4):
                        off = K64LO[tau] + g * nv * 64
                        nc.tensor.matmul(
                            ps[32 * g:32 * g + 32,
                               64 * jlo:64 * (jlo + nv)],
                            fA[0:64, w0 + g + 2, tau, :],
                            wk64[0:64, off:off + nv * 64],
                            start=False,
                            stop=(ti == 5 and g == 3),
                            skip_group_check=True,
                            tile_position=(0, 32 * g),
                        )
                sl = S[:, 512 * wg:512 * wg + 512]
                nc.scalar.activation(sl, psB[:, :], ACT_COPY)
                nc.vector.tensor_add(sl, ps[:], sl)
                if wg % 2 == 1:
                    # eighth-granularity dump keeps the tail short
                    nc.scalar.dma_start(
                        outS[:, 1024 * (wg // 2):1024 * (wg // 2 + 1)],
                        S[:, 1024 * (wg // 2):1024 * (wg // 2 + 1)])
    nc.compile()
    return nc


def _get_nc():
    if "nc" not in _STATE:
        _STATE["nc"] = _build_program()
    return _STATE["nc"]


def _quant_w(a):
    return np.clip(a * WSCALE, -15.0, 15.0).astype(F8E3)


def _prep_inputs(features, weights):
    """Build the 8 per-core input dicts (device layouts)."""
    x = np.asarray(features, dtype=np.float32)
    Wt = np.asarray(weights, dtype=np.float32)

    # w-slot -> real w: last group overlaps (w 58..61)
    widx = list(range(60)) + [58, 59, 60, 61]

    in_maps = []
    for s in STARTS:
        xt = x[:, :, s:s + 10, :].transpose(1, 3, 2, 0)    # [c, w, t, b]
        fA = np.zeros((128, 64, 10, 32), dtype=BF16)
        fA[:64] = xt
        fA[64:, 0:63] = xt[:, 1:]                          # w+1 shift

        Wb = Wt[s:s + 8]                                   # [8, 62, o, c, 3, 3]
        Wsel = Wb[:, widx]                                 # [8, 64slots, o, c, 3, 3]
        WT = Wsel.transpose(4, 5, 3, 0, 1, 2)              # [r, i, c, 8h(j), 64w, o]

        wkf = np.zeros((16, 128, WG_COLS), dtype=np.float32)
        # wr: taps (r, i=d); cols per (tau, g): q -> j=jlo+q, r=tau-j
        wr = wkf[:, :, 0:WR_COLS]
        for tau in TAUS:
            nv, jlo = NV[tau], JLO[tau]
            view = wr[:, :, TBASE[tau]:TBASE[tau + 1]].reshape(
                16, 128, 4, nv, 64)
            for q in range(nv):
                j = jlo + q
                r = tau - j
                for d in range(2):
                    src = WT[r, d][:, j].reshape(CIN, 16, 4, COUT)
                    view[:, d * 64:(d + 1) * 64, :, q, :] = \
                        src.transpose(1, 0, 2, 3)          # [wg, c, g, o]
        # wk64: tap i=2; low taus at partitions 0:64, high at 64:128
        wk64 = wkf[:, :, WR_COLS:WG_COLS]
        for tau in TAUS:
            nv, jlo = NV[tau], JLO[tau]
            if tau in K64LO:
                p0, cb = 0, K64LO[tau]
            else:
                p0, cb = 64, K64HI[tau]
            view = wk64[:, p0:p0 + 64, cb:cb + 4 * nv * 64].reshape(
                16, 64, 4, nv, 64)
            for q in range(nv):
                j = jlo + q
                r = tau - j
                src = WT[r, 2][:, j].reshape(CIN, 16, 4, COUT)
                view[:, :, :, q, :] = src.transpose(1, 0, 2, 3)
        wk = np.ascontiguousarray(_quant_w(wkf))
        in_maps.append({"featA": fA, "wk": wk})
    return in_maps


def _gather(results, bias):
    out = np.zeros((B, COUT, HOUT, WOUT), dtype=np.float32)
    inv = 1.0 / float(WSCALE)
    for core, s in enumerate(STARTS):
        arr = np.asarray(results[core]["outS"]).astype(np.float32) * inv
        # [g, b, wg, j, o] -> [b, o, j, wg, g]
        arr = arr.reshape(4, 32, 16, 8, 64).transpose(1, 4, 3, 2, 0)
        arr = arr.reshape(32, 64, 8, 64)
        out[:, :, s:s + 8, 0:60] = arr[:, :, :, 0:60]
        out[:, :, s:s + 8, 60:62] = arr[:, :, :, 62:64]
    out += np.asarray(bias, dtype=np.float32).transpose(2, 0, 1)[None]
    return out


def _run(in_maps, trace=False, trace_cores=None):
    from concourse.bass_utils import run_bass_kernel_spmd
    nc = _get_nc()
    return run_bass_kernel_spmd(
        nc, in_maps, core_ids=list(range(NCORES)),
        trace=trace, trace_cores=trace_cores,
    )


def kernel(features, weights, bias):
    in_maps = _prep_inputs(features, weights)
    res = _run(in_maps)
    return _gather(res.results, bias)


# revision 15
# speedup vs baseline: 1.2862x; 1.0200x over previous
"""LocallyConnected2d kernel for 8 TRN2 NeuronCores (Bass/Tile).

Problem (hardcoded):
  features [32, 64, 64, 64] f32, weights [62, 62, 64, 64, 3, 3] f32,
  bias [62, 62, 64] f32 -> out [32, 64, 62, 62] f32
  out[b,o,h,w] = sum_{c,i,j} x[b,c,h+i,w+j] * W[h,w,o,c,i,j] + bias[h,w,o]

Strategy (v9 - full 8-row band per work unit):
  - Shard over Hout: 8 cores x 8 output rows (bands [0,8,...,48,54], last two
    overlap; host takes canonical rows from each core).
  - Weights stream as fp8 e3m4 (x2 scale, /2 on host) = 1 B/el -> 18.9 MB/core
    with zero padding waste; activations stay bf16 (mixed-dtype matmul: only
    fp32 operands must be paired). PSUM accumulates fp32. rel err ~0.014.
  - fA layout [128=(c | c shifted w+1), w, t, b]; a [128,32] slice at (w,t) is
    an im2col patch: lower half = x(w), upper = x(w+1).
  - Work unit = one wg (4 w positions), ALL 8 output rows at once: PSUM
    [128, 512] (one full bank): partitions=(4w x 32b) via col tile_position,
    free=(8j x 64 cout). tau = t in 0..9, valid j in [max(0,t-2), min(7,t)].
    * PSUM zeroed OFF the PE (ACT copy-from-zeros / DVE memset); matmuls all
      run flags=0 (has_written: accumulate onto the zeros where stale-set,
      overwrite the zeros where clear). No start=True anywhere.
    * wr MMs (taps i in {0,1}): stationary fA[:, w0+g, tau] K=128, moving
      N=nv*64 <= 192. 40 MMs/wg.
    * tap i=2 is K=64, tau-grouped, split across TWO psum tiles because one
      accumulation group must not mix PE row-tiles (HW limitation, probed):
      taus {0,1,2,5,8,9} via fA lower (w0+g+2) -> main ps (row tile 0);
      taus {3,4,6,7} via the w+1-shifted upper half (w0+g+1, same x column)
      -> psB (row tile 64). psB's j0 columns are never written and stay
      memset-zero. Both wk64 partition halves fully used -> no padding.
    * combine: ACT copies psB -> S slice (bf16), DVE tensor_add(S, ps, S).
  - DMA: 16 wk transfers [128, 9216] fp8 (1.18 MB) on sync; fA w-chunks +
    eighth-granularity outS dumps on scalar.
"""

import numpy as np
import ml_dtypes

BF16 = ml_dtypes.bfloat16
F8E3 = ml_dtypes.float8_e3m4
WSCALE = np.float32(2.0)

B, CIN, COUT = 32, 64, 64
H = W = 64
HOUT = WOUT = 62
NCORES = 8
STARTS = [0, 8, 16, 24, 32, 40, 48, 54]

# tau-group geometry over the full 8-row band: tau = t in 0..9
TAUS = list(range(10))
JLO = [max(0, t - 2) for t in TAUS]
JHI = [min(7, t) for t in TAUS]
NV = [hi - lo + 1 for lo, hi in zip(JLO, JHI)]    # [1,2,3,3,3,3,3,3,2,1]
TBASE = [0]
for t in TAUS:
    TBASE.append(TBASE[-1] + 4 * NV[t] * 64)
WR_COLS = TBASE[-1]                                # 6144
# K64 split: 12 nv-units each side
K64_LOW_TAUS = [0, 1, 2, 5, 8, 9]                  # nv 1+2+3+3+2+1 = 12
K64_HIGH_TAUS = [3, 4, 6, 7]                       # nv 3+3+3+3 = 12
K64LO = {}
off = 0
for t in K64_LOW_TAUS:
    K64LO[t] = off
    off += 4 * NV[t] * 64
K64HI = {}
off = 0
for t in K64_HIGH_TAUS:
    K64HI[t] = off
    off += 4 * NV[t] * 64
K64_COLS = 3072                                    # both halves, 12 units
WG_COLS = WR_COLS + K64_COLS                       # 9216

_STATE = {}


def _build_program():
    import concourse.tile as tile
    from concourse import bacc, mybir

    bf = mybir.dt.bfloat16
    f8 = mybir.dt.float8e3
    f32 = mybir.dt.float32
    ACT_COPY = mybir.ActivationFunctionType.Copy

    nc = bacc.Bacc(None, target_bir_lowering=False)
    featA = nc.dram_tensor("featA", [128, 64, 10, 32], bf, kind="ExternalInput")
    wk_d = nc.dram_tensor("wk", [16, 128, WG_COLS], f8, kind="ExternalInput")
    outS = nc.dram_tensor("outS", [128, 8192], bf, kind="ExternalOutput")

    with tile.TileContext(nc) as tc:
        with tc.tile_pool(name="feat", bufs=1) as fpool, \
             tc.tile_pool(name="wk", bufs=6) as wkpool, \
             tc.tile_pool(name="st", bufs=1) as spool, \
             tc.tile_pool(name="ps", bufs=4, space="PSUM") as pspool, \
             tc.tile_pool(name="psb", bufs=4, space="PSUM") as psbpool:
            fA = fpool.tile([128, 64, 10, 32], bf)
            nc.scalar.dma_start(fA[:, 0:8], featA[:, 0:8])
            nc.scalar.dma_start(fA[:, 8:16], featA[:, 8:16])
            nc.scalar.dma_start(fA[:, 16:24], featA[:, 16:24])
            nc.scalar.dma_start(fA[:, 24:32], featA[:, 24:32])
            nc.scalar.dma_start(fA[:, 32:48], featA[:, 32:48])
            nc.scalar.dma_start(fA[:, 48:64], featA[:, 48:64])
            # zeros for the ACT psum-clearing copies
            zb = fpool.tile([128, 512], bf)
            nc.gpsimd.memset(zb[:], 0.0)
            S = spool.tile([128, 8192], bf)
            for wg in range(16):
                wk = wkpool.tile([128, WG_COLS], f8)
                if wg == 0:
                    # split so the wr slice (consumed first) lands sooner
                    nc.sync.dma_start(wk[:, 0:WR_COLS], wk_d[0][:, 0:WR_COLS])
                    nc.sync.dma_start(wk[:, WR_COLS:WG_COLS],
                                      wk_d[0][:, WR_COLS:WG_COLS])
                else:
                    nc.sync.dma_start(wk[:], wk_d[wg])
                w0 = min(4 * wg, 58)      # last group overlaps: w 58..61
                wr = wk[:, 0:WR_COLS]
                wk64 = wk[:, WR_COLS:WG_COLS]

                ps = pspool.tile([128, 512], f32)
                psB = psbpool.tile([128, 512], f32)
                # Zero PSUM off the PE (see docstring).
                nc.scalar.activation(ps[:, :], zb[:, :], ACT_COPY)
                nc.vector.memset(psB[:, :], 0.0)
                # taps i in {0,1}: K=128 dual-w stationaries
                for tau in TAUS:
                    nv, jlo = NV[tau], JLO[tau]
                    for g in range(4):
                        off = TBASE[tau] + g * nv * 64
                        nc.tensor.matmul(
                            ps[32 * g:32 * g + 32,
                               64 * jlo:64 * (jlo + nv)],
                            fA[:, w0 + g, tau, :],
                            wr[:, off:off + nv * 64],
                            start=False, stop=False,
                            skip_group_check=True,
                            tile_position=(0, 32 * g),
                        )
                # tap i=2, taus {3,4,6,7}: K=64 upper halves -> psB first
                # (psB finishes early so its ACT copy overlaps later MMs)
                for ti, tau in enumerate(K64_HIGH_TAUS):
                    nv, jlo = NV[tau], JLO[tau]
                    for g in range(4):
                        off = K64HI[tau] + g * nv * 64
                        nc.tensor.matmul(
                            psB[32 * g:32 * g + 32,
                                64 * jlo:64 * (jlo + nv)],
                            fA[64:128, w0 + g + 1, tau, :],
                            wk64[64:128, off:off + nv * 64],
                            start=False,
                            stop=(ti == 3 and g == 3),
                            skip_group_check=True,
                            tile_position=(64, 32 * g),
                        )
                # tap i=2, taus {0,1,2,5,8,9}: K=64 lower halves -> main
                for ti, tau in enumerate(K64_LOW_TAUS):
                    nv, jlo = NV[tau], JLO[tau]
                    for g in range(4):
                        off = K64LO[tau] + g * nv * 64
                        nc.tensor.matmul(
                            ps[32 * g:32 * g + 32,
                               64 * jlo:64 * (jlo + nv)],
                            fA[0:64, w0 + g + 2, tau, :],
                            wk64[0:64, off:off + nv * 64],
                            start=False,
                            stop=(ti == 5 and g == 3),
                            skip_group_check=True,
                            tile_position=(0, 32 * g),
                        )
                sl = S[:, 512 * wg:512 * wg + 512]
                nc.scalar.activation(sl, psB[:, :], ACT_COPY)
                nc.vector.tensor_add(sl, ps[:], sl)
                if wg % 2 == 1:
                    # eighth-granularity dump keeps the tail short
                    nc.scalar.dma_start(
                        outS[:, 1024 * (wg // 2):1024 * (wg // 2 + 1)],
                        S[:, 1024 * (wg // 2):1024 * (wg // 2 + 1)])
    nc.compile()
    return nc


def _get_nc():
    if "nc" not in _STATE:
        _STATE["nc"] = _build_program()
    return _STATE["nc"]


def _quant_w(a):
    return np.clip(a * WSCALE, -15.0, 15.0).astype(F8E3)


def _prep_inputs(features, weights):
    """Build the 8 per-core input dicts (device layouts)."""
    x = np.asarray(features, dtype=np.float32)
    Wt = np.asarray(weights, dtype=np.float32)

    # w-slot -> real w: last group overlaps (w 58..61)
    widx = list(range(60)) + [58, 59, 60, 61]

    in_maps = []
    for s in STARTS:
        xt = x[:, :, s:s + 10, :].transpose(1, 3, 2, 0)    # [c, w, t, b]
        fA = np.zeros((128, 64, 10, 32), dtype=BF16)
        fA[:64] = xt
        fA[64:, 0:63] = xt[:, 1:]                          # w+1 shift

        Wb = Wt[s:s + 8]                                   # [8, 62, o, c, 3, 3]
        Wsel = Wb[:, widx]                                 # [8, 64slots, o, c, 3, 3]
        WT = Wsel.transpose(4, 5, 3, 0, 1, 2)              # [r, i, c, 8h(j), 64w, o]

        wkf = np.zeros((16, 128, WG_COLS), dtype=np.float32)
        # wr: taps (r, i=d); cols per (tau, g): q -> j=jlo+q, r=tau-j
        wr = wkf[:, :, 0:WR_COLS]
        for tau in TAUS:
            nv, jlo = NV[tau], JLO[tau]
            view = wr[:, :, TBASE[tau]:TBASE[tau + 1]].reshape(
                16, 128, 4, nv, 64)
            for q in range(nv):
                j = jlo + q
                r = tau - j
                for d in range(2):
                    src = WT[r, d][:, j].reshape(CIN, 16, 4, COUT)
                    view[:, d * 64:(d + 1) * 64, :, q, :] = \
                        src.transpose(1, 0, 2, 3)          # [wg, c, g, o]
        # wk64: tap i=2; low taus at partitions 0:64, high at 64:128
        wk64 = wkf[:, :, WR_COLS:WG_COLS]
        for tau in TAUS:
            nv, jlo = NV[tau], JLO[tau]
            if tau in K64LO:
                p0, cb = 0, K64LO[tau]
            else:
                p0, cb = 64, K64HI[tau]
            view = wk64[:, p0:p0 + 64, cb:cb + 4 * nv * 64].reshape(
                16, 64, 4, nv, 64)
            for q in range(nv):
                j = jlo + q
                r = tau - j
                src = WT[r, 2][:, j].reshape(CIN, 16, 4, COUT)
                view[:, :, :, q, :] = src.transpose(1, 0, 2, 3)
        wk = np.ascontiguousarray(_quant_w(wkf))
        in_maps.append({"featA": fA, "wk": wk})
    return in_maps


def _gather(results, bias):
    out = np.zeros((B, COUT, HOUT, WOUT), dtype=np.float32)
    inv = 1.0 / float(WSCALE)
    for core, s in enumerate(STARTS):
        arr = np.asarray(results[core]["outS"]).astype(np.float32) * inv
        # [g, b, wg, j, o] -> [b, o, j, wg, g]
        arr = arr.reshape(4, 32, 16, 8, 64).transpose(1, 4, 3, 2, 0)
        arr = arr.reshape(32, 64, 8, 64)
        out[:, :, s:s + 8, 0:60] = arr[:, :, :, 0:60]
        out[:, :, s:s + 8, 60:62] = arr[:, :, :, 62:64]
    out += np.asarray(bias, dtype=np.float32).transpose(2, 0, 1)[None]
    return out


def _run(in_maps, trace=False, trace_cores=None):
    from concourse.bass_utils import run_bass_kernel_spmd
    nc = _get_nc()
    return run_bass_kernel_spmd(
        nc, in_maps, core_ids=list(range(NCORES)),
        trace=trace, trace_cores=trace_cores,
    )


def kernel(features, weights, bias):
    in_maps = _prep_inputs(features, weights)
    res = _run(in_maps)
    return _gather(res.results, bias)
